# revision 1
# baseline (speedup 1.0000x reference)
"""Sliding-window causal self-attention (B=4,T=2048,C=1024,H=8,D=128,W=1024)
for 8 Trainium2 NeuronCores.

Sharding: core = (batch b, head-group hg) with 4 heads per group.
Each core computes q/k/v projections for its batch and head group, the
value-embedding gate, rope+rmsnorm, windowed attention, and a partial
output projection over its 512 y-channels.  The host sums the two
partial projections per batch (the "all-reduce after c_proj" done on
host during unshard) and transposes back.

Matmuls run as float32r (TF32-like, ~1.3e-4 max rel err per matmul at
K=1024, 4x faster than fp32); the attention AV / denominator matmuls run
in bf16 on the exp-ed weights.  Softmax is computed without a running
max: rmsnorm bounds |scores| <= sqrt(D) ~= 11.3 so exp never overflows.
"""

import ml_dtypes
import numpy as np

import concourse.bacc as bacc
import concourse.tile as tile
import concourse.mybir as mybir
from concourse.bass_utils import run_bass_kernel_spmd
from concourse.masks import make_identity

F32 = mybir.dt.float32
F32R = mybir.dt.float32r
I32 = mybir.dt.int32
BF16 = mybir.dt.bfloat16
F16 = mybir.dt.float16
AF = mybir.ActivationFunctionType
ALU = mybir.AluOpType

B, T, C, H, D = 4, 2048, 1024, 8, 128
W = 1024          # attention window
HG = 2            # head groups (cores per batch)
HL = H // HG      # heads per core
M = HL * D        # 512 local channels
CH = 128          # t-chunk rows
NCH = T // CH     # 16 chunks
QB = 256          # attention q-block
NQB = T // QB     # 8 q-blocks
CO = C // 128     # 8 contraction chunks
EPS = float(np.finfo(np.float32).eps)
SCALED = float(D) ** -0.5
NEG = -1e30


def build_nc(repeats=1):
    nc = bacc.Bacc(None, target_bir_lowering=False)

    d_xT = nc.dram_tensor("xT", [C, T], F32, kind="ExternalInput")
    d_wqkv = nc.dram_tensor("wqkv", [C, 3 * M], F32, kind="ExternalInput")
    d_wp = nc.dram_tensor("wp", [M, C], F32, kind="ExternalInput")
    d_wg = nc.dram_tensor("wg", [32, HL], F32, kind="ExternalInput")
    d_vet = nc.dram_tensor("vet", [T, M], F32, kind="ExternalInput")
    d_cos = nc.dram_tensor("cosb", [T, 64], F32, kind="ExternalInput")
    d_sin = nc.dram_tensor("sinb", [T, 64], F32, kind="ExternalInput")
    d_masks = nc.dram_tensor("masks", [4, 128, QB], BF16, kind="ExternalInput")
    d_out = nc.dram_tensor("outT", [C, T], F32, kind="ExternalOutput")

    with tile.TileContext(nc) as tc:
        with (
            tc.tile_pool(name="res", bufs=1) as res,
            tc.tile_pool(name="xp", bufs=2) as xp,
            tc.tile_pool(name="vp", bufs=2) as vp,
            tc.tile_pool(name="qk", bufs=2) as qkp,
            tc.tile_pool(name="rope", bufs=3) as rp,
            tc.tile_pool(name="tmp", bufs=2) as tp,
            tc.tile_pool(name="st", bufs=2) as stp,
            tc.tile_pool(name="qt", bufs=2) as qtp,
            tc.tile_pool(name="at", bufs=3) as atp,
            tc.tile_pool(name="yt", bufs=2) as ytp,
            tc.tile_pool(name="oc", bufs=2) as ocp,
            tc.tile_pool(name="pp", bufs=3, space="PSUM") as pp,
            tc.tile_pool(name="sp", bufs=3, space="PSUM") as sp,
            tc.tile_pool(name="dp", bufs=1, space="PSUM") as dp,
            tc.tile_pool(name="yp", bufs=1, space="PSUM") as yp,
        ):
            # ---- resident loads ----
            wqkv = res.tile([128, CO, 3 * M], F32R)
            nc.sync.dma_start(
                wqkv[:],
                d_wqkv.ap().bitcast(F32R).rearrange("(co p) m -> p co m", p=128),
            )
            wp = res.tile([128, HL, C], F32R)
            nc.sync.dma_start(
                wp[:], d_wp.ap().bitcast(F32R).rearrange("(mc p) c -> p mc c", p=128)
            )
            wg = res.tile([32, HL], F32R)
            nc.sync.dma_start(wg[:], d_wg.ap().bitcast(F32R))
            cosb = res.tile([128, NCH, 64], F32)
            nc.sync.dma_start(cosb[:], d_cos.ap().rearrange("(c p) d -> p c d", p=128))
            sinb = res.tile([128, NCH, 64], F32)
            nc.sync.dma_start(sinb[:], d_sin.ap().rearrange("(c p) d -> p c d", p=128))
            masks = res.tile([128, 4, QB], BF16)
            nc.sync.dma_start(masks[:], d_masks.ap().rearrange("k p f -> p k f"))

            nbias = res.tile([128, 1], F32)
            nc.vector.memset(nbias[:], -2.0)
            magic = res.tile([128, 1], I32)
            nc.vector.memset(magic[:], 0x5F3759DF)
            ident = res.tile([128, 128], F32)
            make_identity(nc, ident[:])
            ones16 = res.tile([128, 128], F16)
            nc.vector.memset(ones16[:], 1.0)

            kT = res.tile([128, HL, T], F32R)       # k-hat transposed (d-major)
            vt = res.tile([128, NCH, M], F16)      # v (t-major)
            rkinv = res.tile([128, NCH, HL], F32)   # 1/rms(k) per (t,head)

            def chunk_work(tt, qT_st, riq):
                """projections + gate + rope + norm accumulation for t-chunk tt.

                qT_st: [128, HL, QB] staging for q-hat transposed
                riq:   [128, 2, HL] 1/rms(q)*scaleD for this q-block
                Returns (ropeq, nrm tiles) finished later by apply_chunk.
                """
                ts0 = tt * CH
                xc = xp.tile([128, CO, CH], F32R, tag="xc")
                nc.sync.dma_start(
                    xc[:],
                    d_xT.ap().bitcast(F32R)
                    .rearrange("(co p) t -> p co t", p=128)[:, :, ts0:ts0 + CH],
                )
                vec = vp.tile([128, HL, D], F32, tag="vec")
                nc.sync.dma_start(
                    vec[:],
                    d_vet.ap().rearrange("(c p) (h d) -> p c h d", p=128, h=HL)[:, tt],
                )

                # --- q/k/v projections (t-major) ---
                pq = pp.tile([128, 512], F32, tag="pj")
                pk = pp.tile([128, 512], F32, tag="pj")
                pv = pp.tile([128, 512], F32, tag="pj")
                for mb, ps in ((0, pq), (1, pk), (2, pv)):
                    for co in range(CO):
                        nc.tensor.matmul(
                            ps[:],
                            xc[:, co],
                            wqkv[:, co, mb * M:(mb + 1) * M],
                            start=(co == 0),
                            stop=(co == CO - 1),
                        )
                # gate matmul + sigmoid via exp (stay in exp table set)
                pg = sp.tile([128, QB], F32, tag="sc")
                nc.tensor.matmul(pg[:, :HL], xc[0:32, 0], wg[:], start=True, stop=True)
                eg = stp.tile([128, HL], F32, tag="eg")
                nc.scalar.activation(eg[:], pg[:, :HL], AF.Exp, scale=-1.0)
                nc.vector.tensor_scalar_add(eg[:], eg[:], 1.0)
                gte = stp.tile([128, HL], F32, tag="gt")
                nc.vector.reciprocal(gte[:], eg[:])

                # v += 2*gate*ve  (fold 2x into gateve), write v_t as bf16
                gv = tp.tile([128, HL, D], F32, tag="gv")
                nc.vector.scalar_tensor_tensor(
                    gv[:], vec[:], 2.0,
                    gte[:, :, None].to_broadcast([128, HL, D]),
                    ALU.mult, ALU.mult,
                )
                nc.vector.tensor_tensor(
                    vt[:, tt].rearrange("p (h d) -> p h d", h=HL),
                    pv[:].rearrange("p (h d) -> p h d", h=HL),
                    gv[:], ALU.add,
                )

                # --- copy q,k from psum, rope them ---
                qs = qkp.tile([128, HL, D], F32, tag="qs")
                ks = qkp.tile([128, HL, D], F32, tag="ks")
                nc.scalar.copy(qs[:], pq[:].rearrange("p (h d) -> p h d", h=HL))
                nc.scalar.copy(ks[:], pk[:].rearrange("p (h d) -> p h d", h=HL))

                cosc = cosb[:, tt, None, :].to_broadcast([128, HL, 64])
                sinc = sinb[:, tt, None, :].to_broadcast([128, HL, 64])
                out = []
                for src, tag in ((qs, "rq"), (ks, "rk")):
                    ro = rp.tile([128, HL, D], F32, tag=tag)
                    t1 = tp.tile([128, HL, 64], F32, tag=tag + "t1")
                    t2 = tp.tile([128, HL, 64], F32, tag=tag + "t2")
                    nc.vector.tensor_tensor(ro[:, :, 0:64], src[:, :, 0:64], cosc, ALU.mult)
                    nc.vector.tensor_tensor(ro[:, :, 64:D], src[:, :, 64:D], cosc, ALU.mult)
                    nc.gpsimd.tensor_tensor(t1[:], src[:, :, 64:D], sinc, ALU.mult)
                    nc.gpsimd.tensor_tensor(t2[:], src[:, :, 0:64], sinc, ALU.mult)
                    nc.vector.tensor_tensor(ro[:, :, 0:64], ro[:, :, 0:64], t1[:], ALU.add)
                    nc.vector.tensor_tensor(ro[:, :, 64:D], ro[:, :, 64:D], t2[:], ALU.subtract)
                    out.append(ro)
                return out[0], out[1]

            def norms_chunk(j, ropeq, ropek, nq, nk):
                """accumulate sum(rope^2) over d for each head; j in {0,1}."""
                sq = tp.tile([128, HL, D], F32, tag="sqs")
                for h in range(HL):
                    nc.scalar.activation(
                        sq[:, h], ropeq[:, h], AF.Square,
                        accum_out=nq[:, j, h:h + 1],
                    )
                for h in range(HL):
                    nc.scalar.activation(
                        sq[:, h], ropek[:, h], AF.Square,
                        accum_out=nk[:, j, h:h + 1],
                    )

            def rsqrt_dve(dst, ss, final_scale):
                """dst = final_scale / sqrt(ss), Newton iteration on DVE only."""
                ii = stp.tile([128, 2, HL], I32, tag="ii")
                nc.vector.tensor_scalar(
                    ii[:], ss.bitcast(I32), 1, None, ALU.logical_shift_right
                )
                nc.vector.tensor_tensor(
                    ii[:], magic[:, :, None].to_broadcast([128, 2, HL]), ii[:],
                    ALU.subtract,
                )
                y0 = ii[:].bitcast(F32)
                t = stp.tile([128, 2, HL], F32, tag="nt")
                nc.vector.tensor_tensor(t[:], y0, y0, ALU.mult)
                nc.vector.tensor_tensor(t[:], t[:], ss, ALU.mult)
                nc.vector.tensor_scalar(t[:], t[:], -0.5, 1.5, ALU.mult, ALU.add)
                nc.vector.tensor_tensor(dst, y0, t[:], ALU.mult)
                for it in range(2):
                    last = it == 1
                    nc.vector.tensor_tensor(t[:], dst, dst, ALU.mult)
                    nc.vector.tensor_tensor(t[:], t[:], ss, ALU.mult)
                    s = final_scale if last else 1.0
                    nc.vector.tensor_scalar(
                        t[:], t[:], -0.5 * s, 1.5 * s, ALU.mult, ALU.add
                    )
                    nc.vector.tensor_tensor(dst, dst, t[:], ALU.mult)

            def stats_block(nq, nk, riq, qb):
                ssq = stp.tile([128, 2, HL], F32, tag="ssq")
                ssk = stp.tile([128, 2, HL], F32, tag="ssk")
                nc.vector.tensor_scalar(ssq[:], nq[:], 1.0 / D, EPS, ALU.mult, ALU.add)
                nc.vector.tensor_scalar(ssk[:], nk[:], 1.0 / D, EPS, ALU.mult, ALU.add)
                rsqrt_dve(riq[:], ssq[:], SCALED)
                rsqrt_dve(rkinv[:, 2 * qb:2 * qb + 2, :], ssk[:], 1.0)

            def apply_transpose(tt, j, ropeq, ropek, riq, qT_st):
                """apply q norm, transpose q-hat and k-hat chunks to d-major."""
                qh = tp.tile([128, HL, D], F32, tag="qh")
                for h in range(HL):
                    nc.scalar.activation(
                        qh[:, h], ropeq[:, h], AF.Copy, scale=riq[:, j, h:h + 1]
                    )
                pt = pp.tile([128, 512], F32, tag="pj")
                for h in range(HL):
                    nc.tensor.matmul(
                        pt[:, h * D:(h + 1) * D], qh[:, h], ident[:],
                        is_transpose=True, skip_group_check=True,
                        start=True, stop=True,
                    )
                nc.vector.tensor_copy(
                    qT_st[:, :, j * CH:(j + 1) * CH],
                    pt[:].rearrange("p (h d) -> p h d", h=HL),
                )
                for h in range(HL):
                    nc.scalar.activation(
                        ropek[:, h], ropek[:, h], AF.Copy,
                        scale=rkinv[:, tt, h:h + 1],
                    )
                pt2 = pp.tile([128, 512], F32, tag="pj")
                for h in range(HL):
                    nc.tensor.matmul(
                        pt2[:, h * D:(h + 1) * D], ropek[:, h], ident[:],
                        is_transpose=True, skip_group_check=True,
                        start=True, stop=True,
                    )
                nc.vector.tensor_copy(
                    kT[:, :, tt * CH:(tt + 1) * CH],
                    pt2[:].rearrange("p (h d) -> p h d", h=HL),
                )

            def attention_block(qb, qT_st):
                q0 = qb * QB
                kt0 = max(0, (q0 - W + 1) // 128)
                ktn = (q0 + QB) // 128
                yts = ytp.tile([128, HL, QB], F32R, tag="yts")
                for h in range(HL):
                    ats = []
                    for kp in range(kt0, ktn, 2):
                        ps = sp.tile([128, 2, QB], F32, tag="sc")
                        for i in range(2):
                            kt = kp + i
                            nc.tensor.matmul(
                                ps[:, i], kT[:, h, kt * 128:(kt + 1) * 128],
                                qT_st[:, h], start=True, stop=True,
                                skip_group_check=True,
                            )
                        off = q0 - kp * 128
                        if off == 0:
                            nc.vector.tensor_tensor(ps[:], ps[:], masks[:, 0:2], ALU.add)
                        elif off == W:
                            nc.vector.tensor_tensor(ps[:], ps[:], masks[:, 2:4], ALU.add)
                        at = atp.tile([128, 2, QB], F16, tag="at")
                        nc.scalar.activation(at[:], ps[:], AF.Exp, bias=nbias[:])
                        ats.append((kp, at))
                    dn = dp.tile([128, QB], F32, tag="dn")
                    yv = yp.tile([128, QB], F32, tag="yv")
                    n = ktn - kt0
                    i = 0
                    for kp, at in ats:
                        for j in range(2):
                            kt = kp + j
                            nc.tensor.matmul(
                                dn[:], ones16[:], at[:, j],
                                start=(i == 0), stop=(i == n - 1),
                            )
                            nc.tensor.matmul(
                                yv[:], vt[:, kt, h * D:(h + 1) * D], at[:, j],
                                start=(i == 0), stop=(i == n - 1),
                            )
                            i += 1
                    rc = tp.tile([128, QB], F32, tag="rc")
                    nc.vector.reciprocal(rc[:], dn[:])
                    nc.vector.tensor_tensor(yts[:, h], yv[:], rc[:], ALU.mult)
                return yts

            def proj_block(qb, yts):
                q0 = qb * QB
                outT_r = d_out.ap().rearrange("(cb p) t -> p cb t", p=128)
                for g in range(2):
                    oc = ocp.tile([128, 4, QB], F32, tag="oc")
                    for i in range(4):
                        cb = g * 4 + i
                        po = pp.tile([128, 512], F32, tag="pj")
                        for mc in range(HL):
                            nc.tensor.matmul(
                                po[:, :QB], wp[:, mc, cb * 128:(cb + 1) * 128], yts[:, mc],
                                start=(mc == 0), stop=(mc == HL - 1),
                            )
                        nc.vector.tensor_copy(oc[:, i], po[:, :QB])
                    nc.sync.dma_start(
                        outT_r[:, g * 4:(g + 1) * 4, q0:q0 + QB], oc[:]
                    )

            for _rep in range(repeats):
              for qb in range(NQB):
                  nq = stp.tile([128, 2, HL], F32, tag="nq")
                  nk = stp.tile([128, 2, HL], F32, tag="nk")
                  riq = stp.tile([128, 2, HL], F32, tag="riq")
                  qT_st = qtp.tile([128, HL, QB], F32R, tag="qT")
                  ropes = []
                  for j in range(2):
                      tt = 2 * qb + j
                      rq, rk = chunk_work(tt, qT_st, riq)
                      norms_chunk(j, rq, rk, nq, nk)
                      ropes.append((rq, rk))
                  stats_block(nq, nk, riq, qb)
                  for j in range(2):
                      tt = 2 * qb + j
                      apply_transpose(tt, j, ropes[j][0], ropes[j][1], riq, qT_st)
                  yts = attention_block(qb, qT_st)
                  proj_block(qb, yts)

    nc.finalize()
    return nc


_NC_CACHE = {}


def _get_nc():
    if "nc" not in _NC_CACHE:
        _NC_CACHE["nc"] = build_nc()
    return _NC_CACHE["nc"]


def _make_masks():
    pk = np.arange(128)[:, None]
    fq = np.arange(QB)[None, :]
    m = np.zeros((4, 128, QB), np.float32)
    m[0][~(pk <= fq)] = NEG          # diag, off = 0
    m[1][~(pk <= fq - 128)] = NEG    # diag, off = -128
    m[2][~(pk >= fq + 1)] = NEG      # window, off = W
    m[3][~(pk >= fq - 127)] = NEG    # window, off = W-128
    return m


def kernel(x, ve, cos, sin, Wq, Wk, Wv, Wproj, Wgate, window_size):
    assert int(window_size) == W
    x = np.ascontiguousarray(x, np.float32)
    ve = np.ascontiguousarray(ve, np.float32)
    masks = _make_masks().astype(ml_dtypes.bfloat16)
    cosb = np.ascontiguousarray(cos[0, :, 0, :], np.float32)
    sinb = np.ascontiguousarray(sin[0, :, 0, :], np.float32)

    in_maps = []
    for cid in range(8):
        b, hg = cid // 2, cid % 2
        hs = hg * M
        wqkv = np.ascontiguousarray(
            np.concatenate(
                [Wq[hs:hs + M].T, Wk[hs:hs + M].T, Wv[hs:hs + M].T], axis=1
            ),
            np.float32,
        )
        in_maps.append({
            "xT": np.ascontiguousarray(x[b].T),
            "wqkv": wqkv,
            "wp": np.ascontiguousarray(Wproj[:, hs:hs + M].T, np.float32),
            "wg": np.ascontiguousarray(Wgate[hg * HL:(hg + 1) * HL].T, np.float32),
            "vet": np.ascontiguousarray(ve[b, :, hs:hs + M]),
            "cosb": cosb,
            "sinb": sinb,
            "masks": masks,
        })

    res = run_bass_kernel_spmd(_get_nc(), in_maps, core_ids=list(range(8)))
    _NC_CACHE["last_res"] = res
    out = np.empty((B, T, C), np.float32)
    for b in range(B):
        acc = res.results[2 * b]["outT"] + res.results[2 * b + 1]["outT"]
        out[b] = acc.T
    return out



# revision 16
# speedup vs baseline: 1.6301x; 1.6301x over previous
"""Sliding-window causal self-attention (B=4,T=2048,C=1024,H=8,D=128,W=1024)
for 8 Trainium2 NeuronCores.

Sharding: core = (batch b, head-group hg) with 4 heads per group.  Each core
computes q/k/v projections for its batch and head group, the value-embedding
gate, rope+rmsnorm, windowed attention, and a partial output projection over
its 512 y-channels.  The host sums the two partial projections per batch and
transposes back.

Engine balance: all matmuls bf16/f16 (1 cycle/row).  The elementwise pipeline
runs mostly in bf16 on DVE (packed 2x modes); rope uses a swapped-half AP view
(3 big ops per side); squares and window/causal mask multiplies go to the Pool
engine (pure-SBUF 16-bit ops); rmsnorm rsqrt is a Newton bit-trick on DVE
batched per q-block; PSUM->SBUF staging copies ride the scalar engine, which
stays in activation-table set 0 ({Exp, Copy}) the whole kernel.  Program order
is software-pipelined: projections and rope for block qb interleave with
attention for block qb-1, and score matmuls of head h interleave with the
AV/denominator matmuls of head h-1 so the PE never waits on the exp.
"""

import ml_dtypes
import numpy as np

import concourse.bacc as bacc
import concourse.tile as tile
import concourse.mybir as mybir
from concourse.bass_utils import run_bass_kernel_spmd
from concourse.masks import make_identity

F32 = mybir.dt.float32
I32 = mybir.dt.int32
BF16 = mybir.dt.bfloat16
F16 = mybir.dt.float16
AF = mybir.ActivationFunctionType
ALU = mybir.AluOpType

B, T, C, H, D = 4, 2048, 1024, 8, 128
W = 1024          # attention window
HG = 2            # head groups (cores per batch)
HL = H // HG      # heads per core
M = HL * D        # 512 local channels
CH = 128          # t-chunk rows
NCH = T // CH     # 16 chunks
QB = 256          # attention q-block
NQB = T // QB     # 8 q-blocks
CO = C // 128     # 8 contraction chunks
EPS = float(np.finfo(np.float32).eps)
DEPS = float(D) * EPS


def build_nc(repeats=1, debug=False):
    nc = bacc.Bacc(None, target_bir_lowering=False)

    d_xT = nc.dram_tensor("xT", [C, T], BF16, kind="ExternalInput")
    d_wqkv = nc.dram_tensor("wqkv", [C, 3 * M], BF16, kind="ExternalInput")
    d_wp = nc.dram_tensor("wp", [M, C], BF16, kind="ExternalInput")
    d_wg = nc.dram_tensor("wg", [32, HL], BF16, kind="ExternalInput")
    d_vet = nc.dram_tensor("vet", [T, M], BF16, kind="ExternalInput")
    d_cos = nc.dram_tensor("cose", [T, D], BF16, kind="ExternalInput")
    d_sin = nc.dram_tensor("sine", [T, D], BF16, kind="ExternalInput")
    d_masks = nc.dram_tensor("masks", [4, 128, QB], F16, kind="ExternalInput")
    d_out = nc.dram_tensor("outT", [C, T], F32, kind="ExternalOutput")
    if debug:
        d_dbg_qt = nc.dram_tensor("dbg_qt", [128, HL, QB], BF16, kind="ExternalOutput")
        d_dbg_kt = nc.dram_tensor("dbg_kt", [128, HL, QB], BF16, kind="ExternalOutput")
        d_dbg_at = nc.dram_tensor("dbg_at", [128, 2, QB], F16, kind="ExternalOutput")
        d_dbg_rc = nc.dram_tensor("dbg_rc", [128, QB], F32, kind="ExternalOutput")
        d_dbg_yt = nc.dram_tensor("dbg_yt", [128, HL, QB], BF16, kind="ExternalOutput")
    outT_r = d_out.ap().rearrange("(cb p) t -> p cb t", p=128)

    with tile.TileContext(nc) as tc:
        with (
            tc.tile_pool(name="res", bufs=1) as res,
            tc.tile_pool(name="xp", bufs=2) as xp,
            tc.tile_pool(name="rope", bufs=4) as rp,
            tc.tile_pool(name="tmp", bufs=2) as tp,
            tc.tile_pool(name="st", bufs=2) as stp,
            tc.tile_pool(name="qt", bufs=2) as qtp,
            tc.tile_pool(name="at", bufs=5) as atp,
            tc.tile_pool(name="yt", bufs=2) as ytp,
            tc.tile_pool(name="oc", bufs=2) as ocp,
            tc.tile_pool(name="pp", bufs=2, space="PSUM") as pp,
            tc.tile_pool(name="sp", bufs=2, space="PSUM") as sp,
            tc.tile_pool(name="dp", bufs=1, space="PSUM") as dpp,
            tc.tile_pool(name="yp", bufs=1, space="PSUM") as ypp,
            tc.tile_pool(name="op", bufs=2, space="PSUM") as op,
        ):
            # ---- resident loads ----
            wqkv = res.tile([128, CO, 3 * M], BF16)
            nc.sync.dma_start(
                wqkv[:], d_wqkv.ap().rearrange("(co p) m -> p co m", p=128)
            )
            wp = res.tile([128, HL, C], BF16)
            nc.sync.dma_start(
                wp[:], d_wp.ap().rearrange("(mc p) c -> p mc c", p=128)
            )
            wg = res.tile([32, HL], BF16)
            nc.sync.dma_start(wg[:], d_wg.ap())
            cose = res.tile([128, NCH, D], BF16)
            nc.sync.dma_start(cose[:], d_cos.ap().rearrange("(c p) d -> p c d", p=128))
            sine = res.tile([128, NCH, D], BF16)
            nc.sync.dma_start(sine[:], d_sin.ap().rearrange("(c p) d -> p c d", p=128))
            masks = res.tile([128, 4, QB], F16)
            nc.sync.dma_start(masks[:], d_masks.ap().rearrange("k p f -> p k f"))

            nbias = res.tile([128, 1], F32)
            nc.vector.memset(nbias[:], -2.0)
            magic = res.tile([128, 1], I32)
            nc.vector.memset(magic[:], 0x5F3759DF)
            ident = res.tile([128, 128], BF16)
            make_identity(nc, ident[:])
            ones16 = res.tile([128, 128], F16)
            nc.vector.memset(ones16[:], 1.0)

            kT = res.tile([128, HL, T], BF16)       # k-hat transposed (d-major)
            vt = res.tile([128, NCH, M], F16)       # v (t-major)

            def dma_chunk(tt):
                """Prefetch x / ve slices for chunk tt."""
                ts0 = tt * CH
                xc = xp.tile([128, CO, CH], BF16, tag="xc")
                nc.sync.dma_start(
                    xc[:],
                    d_xT.ap().rearrange("(co p) t -> p co t", p=128)[:, :, ts0:ts0 + CH],
                )
                vec = xp.tile([128, HL, D], BF16, tag="vec")
                nc.sync.dma_start(
                    vec[:],
                    d_vet.ap().rearrange("(c p) (h d) -> p c h d", p=128, h=HL)[:, tt],
                )
                return xc, vec

            def rope_side(tt, src):
                """src: PSUM [128, 512] f32 -> bf16 rope output [128, HL, D]."""
                cb = cose[:, tt, None, :].to_broadcast([128, HL, D])
                sb4 = (
                    sine[:, tt, None, :]
                    .to_broadcast([128, HL, D])
                    .rearrange("p h (j d) -> p h j d", j=2)
                )
                s4 = src[:].rearrange("p (h j d) -> p h j d", h=HL, j=2)
                ro = rp.tile([128, HL, D], BF16, tag="ro")
                t1 = tp.tile([128, HL, D], BF16, tag="t1")
                nc.vector.tensor_tensor(
                    ro[:], src[:].rearrange("p (h d) -> p h d", h=HL), cb, ALU.mult
                )
                nc.vector.tensor_tensor(
                    t1[:].rearrange("p h (j d) -> p h j d", j=2),
                    s4[:, :, ::-1, :], sb4, ALU.mult,
                )
                nc.vector.tensor_tensor(ro[:], ro[:], t1[:], ALU.add)
                return ro

            def chunk_proj(tt, j, xc, vec, n4):
                """projections + gate + v + rope + norms for chunk tt."""
                pq = pp.tile([128, 512], F32, tag="pj")
                for co in range(CO):
                    nc.tensor.matmul(
                        pq[:], xc[:, co], wqkv[:, co, 0:M],
                        start=(co == 0), stop=(co == CO - 1),
                    )
                pg = op.tile([128, 2, QB], F32, tag="oc")
                nc.tensor.matmul(pg[:, 0, :HL], xc[0:32, 0], wg[:], start=True, stop=True)
                pk = pp.tile([128, 512], F32, tag="pj")
                for co in range(CO):
                    nc.tensor.matmul(
                        pk[:], xc[:, co], wqkv[:, co, M:2 * M],
                        start=(co == 0), stop=(co == CO - 1),
                    )
                ropeq = rope_side(tt, pq)      # frees pq for pv reuse
                pv = pp.tile([128, 512], F32, tag="pj")
                for co in range(CO):
                    nc.tensor.matmul(
                        pv[:], xc[:, co], wqkv[:, co, 2 * M:3 * M],
                        start=(co == 0), stop=(co == CO - 1),
                    )
                ropek = rope_side(tt, pk)

                # gate sigmoid: 2/(1+exp(-z))
                eg = stp.tile([128, HL], F32, tag="eg")
                nc.scalar.activation(eg[:], pg[:, 0, :HL], AF.Exp, scale=-1.0)
                nc.vector.tensor_scalar_add(eg[:], eg[:], 1.0)
                gte = stp.tile([128, HL], BF16, tag="gt")
                with nc.allow_low_precision("sigmoid gate, |err|<0.5%"):
                    nc.vector.reciprocal(gte[:], eg[:])
                gv = tp.tile([128, HL, D], BF16, tag="gv")
                nc.vector.scalar_tensor_tensor(
                    gv[:], vec[:], 2.0,
                    gte[:, :, None].to_broadcast([128, HL, D]),
                    ALU.mult, ALU.mult,
                )
                nc.vector.tensor_tensor(
                    vt[:, tt].rearrange("p (h d) -> p h d", h=HL),
                    pv[:].rearrange("p (h d) -> p h d", h=HL),
                    gv[:], ALU.add,
                )

                # norms: sum over d of rope^2 (squares on Pool, reduce on DVE)
                sq = tp.tile([128, 2, HL, D], BF16, tag="sq")
                for s, ro in enumerate((ropeq, ropek)):
                    nc.gpsimd.tensor_tensor(sq[:, s], ro[:], ro[:], ALU.mult)
                    nc.vector.tensor_reduce(
                        n4[:, s, j], sq[:, s], mybir.AxisListType.X, ALU.add
                    )
                return ropeq, ropek

            def rsqrt_dve(dst, ss):
                """dst = 1/sqrt(ss), magic-number Newton on DVE only."""
                sh = ss.shape
                ii = stp.tile(sh, I32, tag="ii")
                nc.vector.tensor_scalar(
                    ii[:], ss.bitcast(I32), 1, None, ALU.logical_shift_right
                )
                nc.vector.tensor_tensor(
                    ii[:], magic[:, :, None, None].to_broadcast(sh), ii[:],
                    ALU.subtract,
                )
                y0 = ii[:].bitcast(F32)
                t = stp.tile(sh, F32, tag="nt")
                nc.vector.tensor_tensor(t[:], y0, y0, ALU.mult)
                nc.vector.tensor_tensor(t[:], t[:], ss, ALU.mult)
                nc.vector.tensor_scalar(t[:], t[:], -0.5, 1.5, ALU.mult, ALU.add)
                y1 = stp.tile(sh, F32, tag="y1")
                nc.vector.tensor_tensor(y1[:], y0, t[:], ALU.mult)
                for it in range(2):
                    nc.vector.tensor_tensor(t[:], y1[:], y1[:], ALU.mult)
                    nc.vector.tensor_tensor(t[:], t[:], ss, ALU.mult)
                    nc.vector.tensor_scalar(t[:], t[:], -0.5, 1.5, ALU.mult, ALU.add)
                    out = dst if it == 1 else y1[:]
                    nc.vector.tensor_tensor(out, y1[:], t[:], ALU.mult)

            def hat_chunk(ropeq, ropek, ri4, j):
                qhat = rp.tile([128, HL, D], BF16, tag="qh")
                nc.vector.tensor_tensor(
                    qhat[:], ropeq[:],
                    ri4[:, 0, j, :, None].to_broadcast([128, HL, D]), ALU.mult,
                )
                khat = rp.tile([128, HL, D], BF16, tag="kh")
                nc.vector.tensor_tensor(
                    khat[:], ropek[:],
                    ri4[:, 1, j, :, None].to_broadcast([128, HL, D]), ALU.mult,
                )
                return qhat, khat

            def transpose_chunk(tt, j, qhat, khat, qT_st):
                pt = sp.tile([128, HL, CH], BF16, tag="sc")
                for h in range(HL):
                    nc.tensor.matmul(
                        pt[:, h], qhat[:, h], ident[:],
                        is_transpose=True, skip_group_check=True,
                        start=True, stop=True,
                    )
                nc.scalar.copy(qT_st[:, :, j * CH:(j + 1) * CH], pt[:])
                pt2 = sp.tile([128, HL, CH], BF16, tag="sc")
                for h in range(HL):
                    nc.tensor.matmul(
                        pt2[:, h], khat[:, h], ident[:],
                        is_transpose=True, skip_group_check=True,
                        start=True, stop=True,
                    )
                nc.scalar.copy(kT[:, :, tt * CH:(tt + 1) * CH], pt2[:])

            def score_head(qb, qT_st, h):
                """score matmuls + exp + mask for head h; returns at tiles."""
                q0 = qb * QB
                kt0 = max(0, (q0 - W + 1) // 128)
                ktn = (q0 + QB) // 128
                ats = []
                for kp in range(kt0, ktn, 2):
                    ps = sp.tile([128, 2, QB], F32, tag="sc")
                    for i in range(2):
                        kt = kp + i
                        nc.tensor.matmul(
                            ps[:, i], kT[:, h, kt * 128:(kt + 1) * 128],
                            qT_st[:, h], start=True, stop=True,
                            skip_group_check=True,
                        )
                    at = atp.tile([128, 2, QB], F16, tag="at")
                    nc.scalar.activation(at[:], ps[:], AF.Exp, bias=nbias[:])
                    off = q0 - kp * 128
                    if off == 0:
                        nc.gpsimd.tensor_tensor(at[:], at[:], masks[:, 0:2], ALU.mult)
                    elif off == W:
                        nc.gpsimd.tensor_tensor(at[:], at[:], masks[:, 2:4], ALU.mult)
                    if debug and qb == 0 and h == 0 and kp == kt0 and _rep[0] == 0:
                        nc.sync.dma_start(d_dbg_at.ap(), at[:])
                    ats.append((kp, at))
                return ats

            def accum_head(qb, h, ats, yts):
                """denominator + AV matmuls + normalization for head h."""
                q0 = qb * QB
                kt0 = max(0, (q0 - W + 1) // 128)
                ktn = (q0 + QB) // 128
                dnt = dpp.tile([128, QB], F32, tag="dn")
                yvt = ypp.tile([128, QB], F32, tag="yv")
                dn, yv = dnt[:], yvt[:]
                n = ktn - kt0
                i = 0
                for kp, at in ats:
                    for j in range(2):
                        kt = kp + j
                        nc.tensor.matmul(
                            dn, ones16[:], at[:, j],
                            start=(i == 0), stop=(i == n - 1),
                            skip_group_check=True,
                        )
                        nc.tensor.matmul(
                            yv, vt[:, kt, h * D:(h + 1) * D], at[:, j],
                            start=(i == 0), stop=(i == n - 1),
                            skip_group_check=True,
                        )
                        i += 1
                rc = tp.tile([128, QB], F32, tag="rc")
                nc.vector.reciprocal(rc[:], dn)
                nc.vector.tensor_tensor(yts[:, h], yv, rc[:], ALU.mult)
                if debug and qb == 0 and h == 3 and _rep[0] == 0:
                    nc.sync.dma_start(d_dbg_rc.ap(), rc[:])
                    nc.sync.dma_start(d_dbg_yt.ap(), yts[:])

            def proj_block(qb, yts):
                q0 = qb * QB
                for g in range(4):
                    po = op.tile([128, 2, QB], F32, tag="oc")
                    for i in range(2):
                        cb = 2 * g + i
                        for mc in range(HL):
                            nc.tensor.matmul(
                                po[:, i], wp[:, mc, cb * 128:(cb + 1) * 128],
                                yts[:, mc],
                                start=(mc == 0), stop=(mc == HL - 1),
                            )
                    oc = ocp.tile([128, 2, QB], F32, tag="st")
                    nc.scalar.copy(oc[:], po[:])
                    nc.sync.dma_start(outT_r[:, 2 * g:2 * g + 2, q0:q0 + QB], oc[:])

            _rep = [0]
            for _rep[0] in range(repeats):
                prev = None  # (qb, qT_st, yts) awaiting attention
                nxt = [dma_chunk(0), dma_chunk(1)]
                for qb in range(NQB + 1):
                    cur, nxt = nxt, []
                    if qb < NQB:
                        qT_st = qtp.tile([128, HL, QB], BF16, tag="qT")
                        n4 = stp.tile([128, 2, 2, HL], F32, tag="n4")
                        r0 = chunk_proj(2 * qb, 0, *cur[0], n4)
                    if prev is not None:
                        pats = [score_head(prev[0], prev[1], 0)]
                        pats.append(score_head(prev[0], prev[1], 1))
                        accum_head(prev[0], 0, pats[0], prev[2])
                    if qb < NQB:
                        r1 = chunk_proj(2 * qb + 1, 1, *cur[1], n4)
                        if qb + 1 < NQB:
                            nxt = [dma_chunk(2 * qb + 2), dma_chunk(2 * qb + 3)]
                    if prev is not None:
                        pats.append(score_head(prev[0], prev[1], 2))
                        accum_head(prev[0], 1, pats[1], prev[2])
                        pats.append(score_head(prev[0], prev[1], 3))
                        accum_head(prev[0], 2, pats[2], prev[2])
                        accum_head(prev[0], 3, pats[3], prev[2])
                        proj_block(prev[0], prev[2])
                    if qb < NQB:
                        nc.vector.tensor_scalar_add(n4[:, 0], n4[:, 0], DEPS)
                        nc.vector.tensor_scalar(
                            n4[:, 1], n4[:, 1], 1.0 / D, EPS, ALU.mult, ALU.add
                        )
                        ri4 = stp.tile([128, 2, 2, HL], F32, tag="ri")
                        rsqrt_dve(ri4[:], n4[:])
                        hat0 = hat_chunk(*r0, ri4, 0)
                        transpose_chunk(2 * qb, 0, *hat0, qT_st)
                        hat1 = hat_chunk(*r1, ri4, 1)
                        transpose_chunk(2 * qb + 1, 1, *hat1, qT_st)
                        if debug and qb == 0 and _rep[0] == 0:
                            nc.sync.dma_start(d_dbg_qt.ap(), qT_st[:])
                            nc.sync.dma_start(d_dbg_kt.ap(), kT[:, :, 0:QB])
                        yts = ytp.tile([128, HL, QB], BF16, tag="yts")
                        prev = (qb, qT_st, yts)

    nc.finalize()
    return nc


_NC_CACHE = {}


def _get_nc():
    if "nc" not in _NC_CACHE:
        _NC_CACHE["nc"] = build_nc()
    return _NC_CACHE["nc"]


def _make_masks():
    pk = np.arange(128)[:, None]
    fq = np.arange(QB)[None, :]
    m = np.zeros((4, 128, QB), np.float16)
    m[0][pk <= fq] = 1.0          # diag, off = 0
    m[1][pk <= fq - 128] = 1.0    # diag, off = -128
    m[2][pk >= fq + 1] = 1.0      # window, off = W
    m[3][pk >= fq - 127] = 1.0    # window, off = W-128
    return m


def kernel(x, ve, cos, sin, Wq, Wk, Wv, Wproj, Wgate, window_size):
    assert int(window_size) == W
    bf = ml_dtypes.bfloat16
    x = np.ascontiguousarray(x, np.float32)
    masks = _make_masks()
    cosb = np.asarray(cos[0, :, 0, :], np.float32)
    sinb = np.asarray(sin[0, :, 0, :], np.float32)
    cose = np.ascontiguousarray(np.concatenate([cosb, cosb], axis=1)).astype(bf)
    sine = np.ascontiguousarray(np.concatenate([sinb, -sinb], axis=1)).astype(bf)

    in_maps = []
    for cid in range(8):
        b, hg = cid // 2, cid % 2
        hs = hg * M
        wqkv = np.ascontiguousarray(
            np.concatenate(
                [Wq[hs:hs + M].T, Wk[hs:hs + M].T, Wv[hs:hs + M].T], axis=1
            )
        ).astype(bf)
        in_maps.append({
            "xT": np.ascontiguousarray(x[b].T).astype(bf),
            "wqkv": wqkv,
            "wp": np.ascontiguousarray(Wproj[:, hs:hs + M].T).astype(bf),
            "wg": np.ascontiguousarray(Wgate[hg * HL:(hg + 1) * HL].T).astype(bf),
            "vet": np.ascontiguousarray(ve[b, :, hs:hs + M]).astype(bf),
            "cose": cose,
            "sine": sine,
            "masks": masks,
        })

    res = run_bass_kernel_spmd(_get_nc(), in_maps, core_ids=list(range(8)))
    _NC_CACHE["last_res"] = res
    out = np.empty((B, T, C), np.float32)
    for b in range(B):
        acc = res.results[2 * b]["outT"] + res.results[2 * b + 1]["outT"]
        out[b] = acc.T
    return out


# revision 17
# speedup vs baseline: 4.2385x; 2.6001x over previous
"""Sliding-window causal self-attention (B=4,T=2048,C=1024,H=8,D=128,W=1024)
for 8 Trainium2 NeuronCores.

Sharding: core = (batch b, head-group hg) with 4 heads per group.  Each core
computes q/k/v projections for its batch and head group, the value-embedding
gate, rope+rmsnorm, windowed attention, and a partial output projection over
its 512 y-channels.  The host sums the two partial projections per batch and
transposes back.

Engine balance: all matmuls bf16/f16 (1 cycle/row).  The elementwise pipeline
runs mostly in bf16 on DVE (packed 2x modes); rope uses a swapped-half AP view
(3 big ops per side); squares and window/causal mask multiplies go to the Pool
engine (pure-SBUF 16-bit ops); rmsnorm rsqrt is a Newton bit-trick on DVE
batched per q-block; PSUM->SBUF staging copies ride the scalar engine, which
stays in activation-table set 0 ({Exp, Copy}) the whole kernel.  Program order
is software-pipelined: projections and rope for block qb interleave with
attention for block qb-1, and score matmuls of head h interleave with the
AV/denominator matmuls of head h-1 so the PE never waits on the exp.
"""

import ml_dtypes
import numpy as np

import concourse.bacc as bacc
import concourse.tile as tile
import concourse.mybir as mybir
from concourse.bass_utils import run_bass_kernel_spmd
from concourse.masks import make_identity

F32 = mybir.dt.float32
I32 = mybir.dt.int32
BF16 = mybir.dt.bfloat16
F16 = mybir.dt.float16
AF = mybir.ActivationFunctionType
ALU = mybir.AluOpType

B, T, C, H, D = 4, 2048, 1024, 8, 128
W = 1024          # attention window
HG = 2            # head groups (cores per batch)
HL = H // HG      # heads per core
M = HL * D        # 512 local channels
CH = 128          # t-chunk rows
NCH = T // CH     # 16 chunks
QB = 256          # attention q-block
NQB = T // QB     # 8 q-blocks
CO = C // 128     # 8 contraction chunks
EPS = float(np.finfo(np.float32).eps)
DEPS = float(D) * EPS


def build_nc(repeats=1, debug=False):
    nc = bacc.Bacc(None, target_bir_lowering=False)

    d_xT = nc.dram_tensor("xT", [C, T], BF16, kind="ExternalInput")
    d_wqkv = nc.dram_tensor("wqkv", [C, 3 * M], BF16, kind="ExternalInput")
    d_wp = nc.dram_tensor("wp", [M, C], BF16, kind="ExternalInput")
    d_wg = nc.dram_tensor("wg", [32, HL], BF16, kind="ExternalInput")
    d_vet = nc.dram_tensor("vet", [T, M], BF16, kind="ExternalInput")
    d_cos = nc.dram_tensor("cose", [T, D], BF16, kind="ExternalInput")
    d_sin = nc.dram_tensor("sine", [T, D], BF16, kind="ExternalInput")
    d_masks = nc.dram_tensor("masks", [4, 128, QB], F16, kind="ExternalInput")
    d_out = nc.dram_tensor("outT", [C, T], F32, kind="ExternalOutput")
    if debug:
        d_dbg_qt = nc.dram_tensor("dbg_qt", [128, HL, QB], BF16, kind="ExternalOutput")
        d_dbg_kt = nc.dram_tensor("dbg_kt", [128, HL, QB], BF16, kind="ExternalOutput")
        d_dbg_at = nc.dram_tensor("dbg_at", [128, 2, QB], F16, kind="ExternalOutput")
        d_dbg_rc = nc.dram_tensor("dbg_rc", [128, QB], F32, kind="ExternalOutput")
        d_dbg_yt = nc.dram_tensor("dbg_yt", [128, HL, QB], BF16, kind="ExternalOutput")
    outT_r = d_out.ap().rearrange("(cb p) t -> p cb t", p=128)

    with tile.TileContext(nc) as tc:
        with (
            tc.tile_pool(name="res", bufs=1) as res,
            tc.tile_pool(name="xp", bufs=2) as xp,
            tc.tile_pool(name="rope", bufs=4) as rp,
            tc.tile_pool(name="tmp", bufs=2) as tp,
            tc.tile_pool(name="st", bufs=2) as stp,
            tc.tile_pool(name="qt", bufs=2) as qtp,
            tc.tile_pool(name="at", bufs=5) as atp,
            tc.tile_pool(name="yt", bufs=2) as ytp,
            tc.tile_pool(name="oc", bufs=2) as ocp,
            tc.tile_pool(name="pp", bufs=2, space="PSUM") as pp,
            tc.tile_pool(name="sp", bufs=2, space="PSUM") as sp,
            tc.tile_pool(name="dp", bufs=1, space="PSUM") as dpp,
            tc.tile_pool(name="yp", bufs=1, space="PSUM") as ypp,
            tc.tile_pool(name="op", bufs=2, space="PSUM") as op,
        ):
            # ---- resident loads ----
            wqkv = res.tile([128, CO, 3 * M], BF16)
            nc.sync.dma_start(
                wqkv[:], d_wqkv.ap().rearrange("(co p) m -> p co m", p=128)
            )
            wp = res.tile([128, HL, C], BF16)
            nc.sync.dma_start(
                wp[:], d_wp.ap().rearrange("(mc p) c -> p mc c", p=128)
            )
            wg = res.tile([32, HL], BF16)
            nc.sync.dma_start(wg[:], d_wg.ap())
            cose = res.tile([128, NCH, D], BF16)
            nc.sync.dma_start(cose[:], d_cos.ap().rearrange("(c p) d -> p c d", p=128))
            sine = res.tile([128, NCH, D], BF16)
            nc.sync.dma_start(sine[:], d_sin.ap().rearrange("(c p) d -> p c d", p=128))
            masks = res.tile([128, 4, QB], F16)
            nc.sync.dma_start(masks[:], d_masks.ap().rearrange("k p f -> p k f"))

            nbias = res.tile([128, 1], F32)
            nc.vector.memset(nbias[:], -2.0)
            magic = res.tile([128, 1], I32)
            nc.vector.memset(magic[:], 0x5F3759DF)
            ident = res.tile([128, 128], BF16)
            make_identity(nc, ident[:])
            ones16 = res.tile([128, 128], F16)
            nc.vector.memset(ones16[:], 1.0)

            kT = res.tile([128, HL, T], BF16)       # k-hat transposed (d-major)
            vt = res.tile([128, NCH, M], F16)       # v (t-major)

            def dma_chunk(tt):
                """Prefetch x / ve slices for chunk tt."""
                ts0 = tt * CH
                xc = xp.tile([128, CO, CH], BF16, tag="xc")
                nc.sync.dma_start(
                    xc[:],
                    d_xT.ap().rearrange("(co p) t -> p co t", p=128)[:, :, ts0:ts0 + CH],
                )
                vec = xp.tile([128, HL, D], BF16, tag="vec")
                nc.sync.dma_start(
                    vec[:],
                    d_vet.ap().rearrange("(c p) (h d) -> p c h d", p=128, h=HL)[:, tt],
                )
                return xc, vec

            def rope_side(tt, src):
                """src: PSUM [128, 512] f32 -> bf16 rope output [128, HL, D]."""
                cb = cose[:, tt, None, :].to_broadcast([128, HL, D])
                sb4 = (
                    sine[:, tt, None, :]
                    .to_broadcast([128, HL, D])
                    .rearrange("p h (j d) -> p h j d", j=2)
                )
                s4 = src[:].rearrange("p (h j d) -> p h j d", h=HL, j=2)
                ro = rp.tile([128, HL, D], BF16, tag="ro")
                t1 = tp.tile([128, HL, D], BF16, tag="t1")
                nc.vector.tensor_tensor(
                    ro[:], src[:].rearrange("p (h d) -> p h d", h=HL), cb, ALU.mult
                )
                nc.vector.tensor_tensor(
                    t1[:].rearrange("p h (j d) -> p h j d", j=2),
                    s4[:, :, ::-1, :], sb4, ALU.mult,
                )
                nc.vector.tensor_tensor(ro[:], ro[:], t1[:], ALU.add)
                return ro

            def chunk_proj(tt, j, xc, vec, n4):
                """projections + gate + v + rope + norms for chunk tt."""
                pq = pp.tile([128, 512], F32, tag="pj")
                for co in range(CO):
                    nc.tensor.matmul(
                        pq[:], xc[:, co], wqkv[:, co, 0:M],
                        start=(co == 0), stop=(co == CO - 1),
                    )
                pg = op.tile([128, 2, QB], F32, tag="oc")
                nc.tensor.matmul(pg[:, 0, :HL], xc[0:32, 0], wg[:], start=True, stop=True)
                pk = pp.tile([128, 512], F32, tag="pj")
                for co in range(CO):
                    nc.tensor.matmul(
                        pk[:], xc[:, co], wqkv[:, co, M:2 * M],
                        start=(co == 0), stop=(co == CO - 1),
                    )
                ropeq = rope_side(tt, pq)      # frees pq for pv reuse
                pv = pp.tile([128, 512], F32, tag="pj")
                for co in range(CO):
                    nc.tensor.matmul(
                        pv[:], xc[:, co], wqkv[:, co, 2 * M:3 * M],
                        start=(co == 0), stop=(co == CO - 1),
                    )
                ropek = rope_side(tt, pk)

                # gate sigmoid: 2/(1+exp(-z))
                eg = stp.tile([128, HL], F32, tag="eg")
                nc.scalar.activation(eg[:], pg[:, 0, :HL], AF.Exp, scale=-1.0)
                nc.vector.tensor_scalar_add(eg[:], eg[:], 1.0)
                gte = stp.tile([128, HL], BF16, tag="gt")
                with nc.allow_low_precision("sigmoid gate, |err|<0.5%"):
                    nc.vector.reciprocal(gte[:], eg[:])
                gv = tp.tile([128, HL, D], BF16, tag="gv")
                nc.vector.scalar_tensor_tensor(
                    gv[:], vec[:], 2.0,
                    gte[:, :, None].to_broadcast([128, HL, D]),
                    ALU.mult, ALU.mult,
                )
                nc.vector.tensor_tensor(
                    vt[:, tt].rearrange("p (h d) -> p h d", h=HL),
                    pv[:].rearrange("p (h d) -> p h d", h=HL),
                    gv[:], ALU.add,
                )

                # norms: sum over d of rope^2 (squares on Pool, reduce on DVE)
                sq = tp.tile([128, 2, HL, D], BF16, tag="sq")
                for s, ro in enumerate((ropeq, ropek)):
                    nc.gpsimd.tensor_tensor(sq[:, s], ro[:], ro[:], ALU.mult)
                    nc.vector.tensor_reduce(
                        n4[:, s, j], sq[:, s], mybir.AxisListType.X, ALU.add
                    )
                return ropeq, ropek

            def rsqrt_dve(dst, ss):
                """dst = 1/sqrt(ss), magic-number Newton on DVE only."""
                sh = ss.shape
                ii = stp.tile(sh, I32, tag="ii")
                nc.vector.tensor_scalar(
                    ii[:], ss.bitcast(I32), 1, None, ALU.logical_shift_right
                )
                nc.vector.tensor_tensor(
                    ii[:], magic[:, :, None, None].to_broadcast(sh), ii[:],
                    ALU.subtract,
                )
                y0 = ii[:].bitcast(F32)
                t = stp.tile(sh, F32, tag="nt")
                nc.vector.tensor_tensor(t[:], y0, y0, ALU.mult)
                nc.vector.tensor_tensor(t[:], t[:], ss, ALU.mult)
                nc.vector.tensor_scalar(t[:], t[:], -0.5, 1.5, ALU.mult, ALU.add)
                y1 = stp.tile(sh, F32, tag="y1")
                nc.vector.tensor_tensor(y1[:], y0, t[:], ALU.mult)
                nc.vector.tensor_tensor(t[:], y1[:], y1[:], ALU.mult)
                nc.vector.tensor_tensor(t[:], t[:], ss, ALU.mult)
                nc.vector.tensor_scalar(t[:], t[:], -0.5, 1.5, ALU.mult, ALU.add)
                nc.vector.tensor_tensor(dst, y1[:], t[:], ALU.mult)

            def hat_chunk(ropeq, ropek, ri4, j):
                qhat = rp.tile([128, HL, D], BF16, tag="qh")
                nc.vector.tensor_tensor(
                    qhat[:], ropeq[:],
                    ri4[:, 0, j, :, None].to_broadcast([128, HL, D]), ALU.mult,
                )
                khat = rp.tile([128, HL, D], BF16, tag="kh")
                nc.vector.tensor_tensor(
                    khat[:], ropek[:],
                    ri4[:, 1, j, :, None].to_broadcast([128, HL, D]), ALU.mult,
                )
                return qhat, khat

            def transpose_chunk(tt, j, qhat, khat, qT_st):
                pt = sp.tile([128, HL, CH], BF16, tag="sc")
                for h in range(HL):
                    nc.tensor.matmul(
                        pt[:, h], qhat[:, h], ident[:],
                        is_transpose=True, skip_group_check=True,
                        start=True, stop=True,
                    )
                nc.vector.tensor_copy(qT_st[:, :, j * CH:(j + 1) * CH], pt[:])
                pt2 = sp.tile([128, HL, CH], BF16, tag="sc")
                for h in range(HL):
                    nc.tensor.matmul(
                        pt2[:, h], khat[:, h], ident[:],
                        is_transpose=True, skip_group_check=True,
                        start=True, stop=True,
                    )
                nc.vector.tensor_copy(kT[:, :, tt * CH:(tt + 1) * CH], pt2[:])

            def score_head(qb, qT_st, h):
                """score matmuls + exp + mask for head h; returns at tiles."""
                q0 = qb * QB
                kt0 = max(0, (q0 - W + 1) // 128)
                ktn = (q0 + QB) // 128
                ats = []
                for kp in range(kt0, ktn, 2):
                    ps = sp.tile([128, 2, QB], F32, tag="sc")
                    for i in range(2):
                        kt = kp + i
                        nc.tensor.matmul(
                            ps[:, i], kT[:, h, kt * 128:(kt + 1) * 128],
                            qT_st[:, h], start=True, stop=True,
                            skip_group_check=True,
                        )
                    at = atp.tile([128, 2, QB], F16, tag="at")
                    nc.scalar.activation(at[:], ps[:], AF.Exp, bias=nbias[:])
                    off = q0 - kp * 128
                    if off == 0:
                        nc.gpsimd.tensor_tensor(at[:], at[:], masks[:, 0:2], ALU.mult)
                    elif off == W:
                        nc.gpsimd.tensor_tensor(at[:], at[:], masks[:, 2:4], ALU.mult)
                    if debug and qb == 0 and h == 0 and kp == kt0 and _rep[0] == 0:
                        nc.sync.dma_start(d_dbg_at.ap(), at[:])
                    ats.append((kp, at))
                return ats

            def accum_head(qb, h, ats, yts):
                """denominator + AV matmuls + normalization for head h."""
                q0 = qb * QB
                kt0 = max(0, (q0 - W + 1) // 128)
                ktn = (q0 + QB) // 128
                dnt = dpp.tile([128, QB], F32, tag="dn")
                yvt = ypp.tile([128, QB], F32, tag="yv")
                dn, yv = dnt[:], yvt[:]
                n = ktn - kt0
                i = 0
                for kp, at in ats:
                    for j in range(2):
                        kt = kp + j
                        nc.tensor.matmul(
                            dn, ones16[:], at[:, j],
                            start=(i == 0), stop=(i == n - 1),
                            skip_group_check=True,
                        )
                        nc.tensor.matmul(
                            yv, vt[:, kt, h * D:(h + 1) * D], at[:, j],
                            start=(i == 0), stop=(i == n - 1),
                            skip_group_check=True,
                        )
                        i += 1
                rc = tp.tile([128, QB], F32, tag="rc")
                nc.vector.reciprocal_approx_fast(rc[:], dn)
                nc.vector.tensor_tensor(yts[:, h], yv, rc[:], ALU.mult)
                if debug and qb == 0 and h == 3 and _rep[0] == 0:
                    nc.sync.dma_start(d_dbg_rc.ap(), rc[:])
                    nc.sync.dma_start(d_dbg_yt.ap(), yts[:])

            def proj_block(qb, yts):
                q0 = qb * QB
                for g in range(4):
                    po = op.tile([128, 2, QB], F32, tag="oc")
                    for i in range(2):
                        cb = 2 * g + i
                        for mc in range(HL):
                            nc.tensor.matmul(
                                po[:, i], wp[:, mc, cb * 128:(cb + 1) * 128],
                                yts[:, mc],
                                start=(mc == 0), stop=(mc == HL - 1),
                            )
                    oc = ocp.tile([128, 2, QB], F32, tag="st")
                    nc.scalar.copy(oc[:], po[:])
                    nc.sync.dma_start(outT_r[:, 2 * g:2 * g + 2, q0:q0 + QB], oc[:])

            _rep = [0]
            for _rep[0] in range(repeats):
                prev = None  # (qb, qT_st, yts) awaiting attention
                nxt = [dma_chunk(0), dma_chunk(1)]
                for qb in range(NQB + 1):
                    cur, nxt = nxt, []
                    if qb < NQB:
                        qT_st = qtp.tile([128, HL, QB], BF16, tag="qT")
                        n4 = stp.tile([128, 2, 2, HL], F32, tag="n4")
                        r0 = chunk_proj(2 * qb, 0, *cur[0], n4)
                    if prev is not None:
                        pats = [score_head(prev[0], prev[1], 0)]
                        pats.append(score_head(prev[0], prev[1], 1))
                        accum_head(prev[0], 0, pats[0], prev[2])
                    if qb < NQB:
                        r1 = chunk_proj(2 * qb + 1, 1, *cur[1], n4)
                        if qb + 1 < NQB:
                            nxt = [dma_chunk(2 * qb + 2), dma_chunk(2 * qb + 3)]
                    if prev is not None:
                        pats.append(score_head(prev[0], prev[1], 2))
                        accum_head(prev[0], 1, pats[1], prev[2])
                        pats.append(score_head(prev[0], prev[1], 3))
                        accum_head(prev[0], 2, pats[2], prev[2])
                        accum_head(prev[0], 3, pats[3], prev[2])
                        proj_block(prev[0], prev[2])
                    if qb < NQB:
                        nc.vector.tensor_scalar_add(n4[:, 0], n4[:, 0], DEPS)
                        nc.vector.tensor_scalar(
                            n4[:, 1], n4[:, 1], 1.0 / D, EPS, ALU.mult, ALU.add
                        )
                        ri4 = stp.tile([128, 2, 2, HL], BF16, tag="ri")
                        rsqrt_dve(ri4[:], n4[:])
                        hat0 = hat_chunk(*r0, ri4, 0)
                        transpose_chunk(2 * qb, 0, *hat0, qT_st)
                        hat1 = hat_chunk(*r1, ri4, 1)
                        transpose_chunk(2 * qb + 1, 1, *hat1, qT_st)
                        if debug and qb == 0 and _rep[0] == 0:
                            nc.sync.dma_start(d_dbg_qt.ap(), qT_st[:])
                            nc.sync.dma_start(d_dbg_kt.ap(), kT[:, :, 0:QB])
                        yts = ytp.tile([128, HL, QB], BF16, tag="yts")
                        prev = (qb, qT_st, yts)

    nc.finalize()
    return nc


_NC_CACHE = {}


def _get_nc():
    if "nc" not in _NC_CACHE:
        _NC_CACHE["nc"] = build_nc()
    return _NC_CACHE["nc"]


def _make_masks():
    pk = np.arange(128)[:, None]
    fq = np.arange(QB)[None, :]
    m = np.zeros((4, 128, QB), np.float16)
    m[0][pk <= fq] = 1.0          # diag, off = 0
    m[1][pk <= fq - 128] = 1.0    # diag, off = -128
    m[2][pk >= fq + 1] = 1.0      # window, off = W
    m[3][pk >= fq - 127] = 1.0    # window, off = W-128
    return m


def kernel(x, ve, cos, sin, Wq, Wk, Wv, Wproj, Wgate, window_size):
    assert int(window_size) == W
    bf = ml_dtypes.bfloat16
    x = np.ascontiguousarray(x, np.float32)
    masks = _make_masks()
    cosb = np.asarray(cos[0, :, 0, :], np.float32)
    sinb = np.asarray(sin[0, :, 0, :], np.float32)
    cose = np.ascontiguousarray(np.concatenate([cosb, cosb], axis=1)).astype(bf)
    sine = np.ascontiguousarray(np.concatenate([sinb, -sinb], axis=1)).astype(bf)

    in_maps = []
    for cid in range(8):
        b, hg = cid // 2, cid % 2
        hs = hg * M
        wqkv = np.ascontiguousarray(
            np.concatenate(
                [Wq[hs:hs + M].T, Wk[hs:hs + M].T, Wv[hs:hs + M].T], axis=1
            )
        ).astype(bf)
        in_maps.append({
            "xT": np.ascontiguousarray(x[b].T).astype(bf),
            "wqkv": wqkv,
            "wp": np.ascontiguousarray(Wproj[:, hs:hs + M].T).astype(bf),
            "wg": np.ascontiguousarray(Wgate[hg * HL:(hg + 1) * HL].T).astype(bf),
            "vet": np.ascontiguousarray(ve[b, :, hs:hs + M]).astype(bf),
            "cose": cose,
            "sine": sine,
            "masks": masks,
        })

    res = run_bass_kernel_spmd(_get_nc(), in_maps, core_ids=list(range(8)))
    _NC_CACHE["last_res"] = res
    out = np.empty((B, T, C), np.float32)
    for b in range(B):
        acc = res.results[2 * b]["outT"] + res.results[2 * b + 1]["outT"]
        out[b] = acc.T
    return out
